# revision 12
# baseline (speedup 1.0000x reference)
# Trainium2 Bass kernel for nn_CausalGraphAttn (B=4, N=2048, D=1024, H=16).
#
# Sharding: 8 cores = 4 batches x 2 head-groups. Each head-group has 8 of the
# 16 heads, chosen so every core gets exactly 2 causal (graph-masked) heads —
# this keeps the SPMD program identical across cores. Each core computes
# LN -> qkv (its heads) -> attention -> partial out-projection; the host sums
# the two partials per batch. Residual + output bias ride on the even core of
# each pair (odd cores get zero tensors), so the device does the full compute.
#
# All matmuls run as float32r (full PE rate at N>=512, ~19-bit mantissa).
# Attention works in transposed space: S^T[k,q] = k·q, exp on ScalarE straight
# from PSUM, PV with an augmented ones-column in V producing [hd+1, q] so the
# softmax denominator falls out of the same matmul. The graph mask is a 0/1
# bf16 multiply on VectorE after the exp.
#
# SBUF is managed with two big 8-tile pools whose tiles are reused across
# phases (xt->qkT, hT->otn/et/mask/recip-rows) so pool lifetimes nest LIFO.

import sys
import numpy as np

if "/opt/trn_rl_repo" not in sys.path:
    sys.path.insert(0, "/opt/trn_rl_repo")

B = 4
N = 2048
D = 1024
HD = 64
LN_EPS = 1e-5
N_CORES = 8
NKT = N // 128          # 16 k/token tiles
NDT = D // 128          # 8 feature tiles
HEADS_G0 = [0, 1, 4, 5, 6, 7, 8, 9]
HEADS_G1 = [2, 3, 10, 11, 12, 13, 14, 15]

_CACHE = {}


def _build():
    if "nc" in _CACHE:
        return _CACHE["nc"]
    from contextlib import ExitStack
    import concourse.bass as bass  # noqa: F401
    import concourse.mybir as mybir
    import concourse.tile as tile
    from concourse import bacc

    f32 = mybir.dt.float32
    f32r = mybir.dt.float32r
    bf16 = mybir.dt.bfloat16
    EXP = mybir.ActivationFunctionType.Exp
    SQUARE = mybir.ActivationFunctionType.Square
    SQRT = mybir.ActivationFunctionType.Sqrt

    nc = bacc.Bacc("TRN2", target_bir_lowering=False, debug=False,
                   num_devices=N_CORES)

    xT_d = nc.dram_tensor("xT", [D, N], f32, kind="ExternalInput").ap()
    xres_d = nc.dram_tensor("xres", [N, D], f32, kind="ExternalInput").ap()
    wqkT_d = nc.dram_tensor("wqkT", [D, D], f32, kind="ExternalInput").ap()
    wvT_d = nc.dram_tensor("wvT", [D, 512], f32, kind="ExternalInput").ap()
    woT_d = nc.dram_tensor("woT", [512, D], f32, kind="ExternalInput").ap()
    bqk_d = nc.dram_tensor("bqk", [D, 1], f32, kind="ExternalInput").ap()
    bv_d = nc.dram_tensor("bv", [1, 512], f32, kind="ExternalInput").ap()
    bo_d = nc.dram_tensor("bo", [1, D], f32, kind="ExternalInput").ap()
    mask_d = nc.dram_tensor("maskT", [N, N], bf16, kind="ExternalInput").ap()
    y_d = nc.dram_tensor("y", [N, D], f32, kind="ExternalOutput").ap()

    with tile.TileContext(nc) as tc, ExitStack() as top:
        const = top.enter_context(tc.tile_pool(name="const", bufs=1))
        ones_f = const.tile([128, 8], f32, name="ones_f", tag="ones_f")
        nc.vector.memset(ones_f, 1.0)
        ones_col = const.tile([128, 1], f32r, name="ones_col", tag="ones_col")
        nc.vector.tensor_copy(ones_col, ones_f[:, 0:1])
        ones_fr = const.tile([1, 128], f32, name="ones_fr", tag="ones_fr")
        nc.vector.memset(ones_fr, 1.0)
        ones_row = const.tile([1, 128], f32r, name="ones_row", tag="ones_row")
        nc.vector.tensor_copy(ones_row, ones_fr)
        bqk_sb = const.tile([128, 8], f32, name="bqk_sb", tag="bqk_sb")
        for m in range(NDT):
            nc.sync.dma_start(out=bqk_sb[:, m:m + 1],
                              in_=bqk_d[m * 128:(m + 1) * 128, :])
        bv_sb = const.tile([1, 512], f32r, name="bv_sb", tag="bv_sb")
        nc.sync.dma_start(out=bv_sb, in_=bv_d.bitcast(f32r))
        bo_sb = const.tile([1, D], f32r, name="bo_sb", tag="bo_sb")
        nc.sync.dma_start(out=bo_sb, in_=bo_d.bitcast(f32r))
        eps_sb = const.tile([1, 1], f32, name="eps_sb", tag="eps_sb")
        nc.vector.memset(eps_sb, LN_EPS)

        # Two big pools of 8x[128, 2048] f32r tiles, reused across phases.
        bigA_pool = top.enter_context(tc.tile_pool(name="bigA", bufs=1))
        bigA = [bigA_pool.tile([128, N], f32r, name=f"bigA{i}", tag=f"bigA{i}")
                for i in range(NDT)]
        bigB_pool = top.enter_context(tc.tile_pool(name="bigB", bufs=1))
        bigB = [bigB_pool.tile([128, N], f32r, name=f"bigB{i}", tag=f"bigB{i}")
                for i in range(NDT)]
        xt = bigA      # phase B/C
        qkT = bigA     # phase D..F (overwrites xt)
        hT = bigB      # phase C..E
        otn = bigB[0:4]  # phase F..G (overwrites hT)

        # ---- Phase B: LN stats via ones-matmuls ----
        for i in range(NDT):
            nc.sync.dma_start(
                out=xt[i], in_=xT_d[i * 128:(i + 1) * 128, :].bitcast(f32r))
        with ExitStack() as ph:
            bc_pool = ph.enter_context(tc.tile_pool(name="bcp", bufs=1))
            rstd_bc = bc_pool.tile([128, N], f32, name="rstd_bc", tag="rstd_bc")
            ms_bc = bc_pool.tile([128, N], f32, name="ms_bc", tag="ms_bc")
            with ExitStack() as st_blk:
                pss = st_blk.enter_context(
                    tc.tile_pool(name="ps_stat", bufs=1, space="PSUM"))
                sum_ps = pss.tile([1, N], f32, name="sum_ps", tag="sum_ps")
                ssq_ps = pss.tile([1, N], f32, name="ssq_ps", tag="ssq_ps")
                with tc.tile_pool(name="sqp", bufs=2) as sq_pool:
                    for i in range(NDT):
                        sq = sq_pool.tile([128, N], f32r, name="sq", tag="sq")
                        nc.scalar.activation(sq, xt[i].bitcast(f32), SQUARE)
                        for s in range(4):
                            sl = slice(s * 512, (s + 1) * 512)
                            nc.tensor.matmul(sum_ps[:, sl], ones_col,
                                             xt[i][:, sl],
                                             start=(i == 0), stop=(i == NDT - 1))
                            nc.tensor.matmul(ssq_ps[:, sl], ones_col, sq[:, sl],
                                             start=(i == 0), stop=(i == NDT - 1))
                keep_pool = st_blk.enter_context(
                    tc.tile_pool(name="rowk", bufs=1))
                rot_pool = st_blk.enter_context(
                    tc.tile_pool(name="rowr", bufs=2))
                mu = keep_pool.tile([1, N], f32, name="mu", tag="mu")
                nc.vector.tensor_scalar_mul(mu, sum_ps, 1.0 / D)
                musq = rot_pool.tile([1, N], f32, name="musq", tag="rt")
                nc.vector.tensor_mul(musq, mu, mu)
                var = rot_pool.tile([1, N], f32, name="var", tag="rt")
                nc.vector.scalar_tensor_tensor(
                    var, ssq_ps, 1.0 / D, musq,
                    op0=mybir.AluOpType.mult, op1=mybir.AluOpType.subtract)
                sd = rot_pool.tile([1, N], f32, name="sd", tag="rt")
                nc.scalar.activation(sd, var, SQRT, bias=eps_sb)
                rstd = keep_pool.tile([1, N], f32, name="rstd", tag="rstd")
                scr = rot_pool.tile([1, N], f32, name="scr", tag="rt")
                nc.vector.reciprocal_approx_accurate(rstd, sd, scr)
                ms = rot_pool.tile([1, N], f32, name="ms", tag="rt")
                nc.vector.tensor_mul(ms, mu, rstd)
                nc.gpsimd.partition_broadcast(rstd_bc, rstd)
                nc.gpsimd.partition_broadcast(ms_bc, ms)
            # ---- Phase C: hT = xT*rstd - mu*rstd ----
            with tc.tile_pool(name="tmpp", bufs=2) as tmp_pool:
                for i in range(NDT):
                    t = tmp_pool.tile([128, N], f32, name="tmp", tag="tmp")
                    nc.vector.tensor_mul(t, xt[i].bitcast(f32), rstd_bc)
                    nc.vector.tensor_sub(hT[i], t, ms_bc)

        # ---- Phase D: qkT = wqk @ hT + bqk (streamed weights) ----
        with ExitStack() as ph:
            wq_pool = ph.enter_context(tc.tile_pool(name="wqp", bufs=3))
            psq_pool = ph.enter_context(
                tc.tile_pool(name="ps_q", bufs=2, space="PSUM"))
            for m in range(NDT):
                ps = psq_pool.tile([128, N], f32, name="psq", tag="psq")
                for i in range(NDT):
                    w = wq_pool.tile([128, 128], f32r, name="wq", tag="wq")
                    nc.sync.dma_start(
                        out=w, in_=wqkT_d[i * 128:(i + 1) * 128,
                                          m * 128:(m + 1) * 128].bitcast(f32r))
                    for s in range(4):
                        sl = slice(s * 512, (s + 1) * 512)
                        nc.tensor.matmul(ps[:, sl], w, hT[i][:, sl],
                                         start=(i == 0), stop=(i == NDT - 1))
                nc.vector.tensor_scalar_add(qkT[m], ps, bqk_sb[:, m:m + 1])

        # ---- Phase E: v = hT.T @ wvT + bv, packed as v_aug with ones col ----
        with ExitStack() as ph:
            vaug_pool = ph.enter_context(tc.tile_pool(name="vaugp", bufs=1))
            vaug = [vaug_pool.tile([128, 8 * 65], f32r, name=f"vaug{t}",
                                   tag=f"vaug{t}") for t in range(NKT)]
            with ExitStack() as inner:
                wv_pool = inner.enter_context(tc.tile_pool(name="wvp", bufs=1))
                wv = [wv_pool.tile([128, 512], f32r, name=f"wv{i}",
                                   tag=f"wv{i}") for i in range(NDT)]
                for i in range(NDT):
                    nc.sync.dma_start(
                        out=wv[i],
                        in_=wvT_d[i * 128:(i + 1) * 128, :].bitcast(f32r))
                psv_pool = inner.enter_context(
                    tc.tile_pool(name="ps_v", bufs=3, space="PSUM"))
                for t in range(NKT):
                    ps = psv_pool.tile([128, 512], f32, name="psv", tag="psv")
                    for i in range(NDT):
                        nc.tensor.matmul(ps, hT[i][:, t * 128:(t + 1) * 128],
                                         wv[i], start=(i == 0), stop=False)
                    nc.tensor.matmul(ps, ones_row, bv_sb,
                                     start=False, stop=True)
                    v3 = vaug[t].rearrange("p (h e) -> p h e", h=8)
                    nc.vector.tensor_copy(v3[:, :, 0:64],
                                          ps.rearrange("p (h e) -> p h e", h=8))
                    nc.vector.tensor_copy(v3[:, :, 64:65], ones_f.unsqueeze(-1))

            # ---- Phase F: attention, head-sequential ----
            # bigB tiles 0-3 become otn, 4-5 rotate as E=exp(S^T); the bf16
            # mask (two halves) and the r/recip/broadcast rows get their own
            # small pools (the BIR verifier forbids non-f32r writers on
            # locations an f32r matmul reads, so they can't reuse bigB).
            with ExitStack() as inner:
                mk_pool = inner.enter_context(tc.tile_pool(name="mkp", bufs=1))
                mkv = mk_pool.tile([128, 2 * N], bf16, name="mkv", tag="mkv")
                rr_pool = inner.enter_context(tc.tile_pool(name="rrp", bufs=1))
                rtile = rr_pool.tile([128, N], f32, name="rtile", tag="rtile")
                rsb = rtile[0:1, :]
                rr = rtile[32:33, :]
                rrb = rtile[64:128, :]
                pst_pool = inner.enter_context(
                    tc.tile_pool(name="ps_st", bufs=2, space="PSUM"))
                pso_pool = inner.enter_context(
                    tc.tile_pool(name="ps_ot", bufs=1, space="PSUM"))
                for h in range(8):
                    tq = h // 2
                    tk = 4 + h // 2
                    base = 64 * (h % 2)
                    ot = pso_pool.tile([65, N], f32, name="ot", tag="ot")
                    for kt in range(NKT):
                        et = bigB[4 + kt % 2]
                        if h < 2:
                            mk = mkv[:, (kt % 2) * N:(kt % 2 + 1) * N]
                            nc.sync.dma_start(
                                out=mk, in_=mask_d[kt * 128:(kt + 1) * 128, :])
                        for qh in range(2):
                            st = pst_pool.tile([128, 1024], f32,
                                               name="st", tag="st")
                            for qs in range(2):
                                qo = qh * 1024 + qs * 512
                                nc.tensor.matmul(
                                    st[:, qs * 512:(qs + 1) * 512],
                                    qkT[tk][base:base + 64,
                                            kt * 128:(kt + 1) * 128],
                                    qkT[tq][base:base + 64, qo:qo + 512],
                                    start=True, stop=True)
                            esl = et[:, qh * 1024:(qh + 1) * 1024]
                            nc.scalar.activation(esl, st, EXP, scale=0.125)
                            if h < 2:
                                nc.vector.tensor_mul(
                                    esl, esl,
                                    mk[:, qh * 1024:(qh + 1) * 1024])
                            for qs in range(2):
                                qo = qh * 1024 + qs * 512
                                nc.tensor.matmul(
                                    ot[:, qo:qo + 512],
                                    vaug[kt][:, h * 65:h * 65 + 65],
                                    et[:, qo:qo + 512],
                                    start=(kt == 0), stop=(kt == NKT - 1))
                    nc.vector.tensor_copy(rsb, ot[64:65, :])
                    nc.vector.reciprocal_approx_fast(rr, rsb)
                    nc.gpsimd.partition_broadcast(rrb, rr)
                    nc.vector.tensor_mul(otn[tq][base:base + 64, :],
                                         ot[0:64, :], rrb)

        # ---- Phase G: y = otn.T @ woT + bo + xres ----
        with ExitStack() as ph:
            wo_pool = ph.enter_context(tc.tile_pool(name="wop", bufs=1))
            wo = [wo_pool.tile([128, D], f32r, name=f"wo{f}", tag=f"wo{f}")
                  for f in range(4)]
            for f in range(4):
                nc.sync.dma_start(
                    out=wo[f], in_=woT_d[f * 128:(f + 1) * 128, :].bitcast(f32r))
            xr_pool = ph.enter_context(tc.tile_pool(name="xrp", bufs=3))
            y_pool = ph.enter_context(tc.tile_pool(name="yp", bufs=3))
            psy_pool = ph.enter_context(
                tc.tile_pool(name="ps_y", bufs=2, space="PSUM"))
            for t in range(NKT):
                xr = xr_pool.tile([128, D], f32, name="xr", tag="xr")
                nc.sync.dma_start(out=xr, in_=xres_d[t * 128:(t + 1) * 128, :])
                ps = psy_pool.tile([128, D], f32, name="psy", tag="psy")
                for s in range(2):
                    sl = slice(s * 512, (s + 1) * 512)
                    for f in range(4):
                        nc.tensor.matmul(ps[:, sl],
                                         otn[f][:, t * 128:(t + 1) * 128],
                                         wo[f][:, sl],
                                         start=(f == 0), stop=False)
                    nc.tensor.matmul(ps[:, sl], ones_row, bo_sb[:, sl],
                                     start=False, stop=True)
                yt = y_pool.tile([128, D], f32, name="yt", tag="yt")
                nc.vector.tensor_add(yt, ps, xr)
                nc.sync.dma_start(out=y_d[t * 128:(t + 1) * 128, :], in_=yt)

    nc.compile()
    _CACHE["nc"] = nc
    return nc


def _prep_inputs(x, adj, w_qkv, w_out, b_out, ln_gamma, ln_beta):
    import ml_dtypes
    x = np.asarray(x, np.float32)
    adj = np.asarray(adj, np.float32)
    w_qkv = np.asarray(w_qkv, np.float32)
    w_out = np.asarray(w_out, np.float32)
    b_out = np.asarray(b_out, np.float32)
    g = np.asarray(ln_gamma, np.float32)
    bt = np.asarray(ln_beta, np.float32)

    maskT = (adj >= 0.1).astype(ml_dtypes.bfloat16)
    zeros_res = np.zeros((N, D), np.float32)
    zeros_bo = np.zeros((1, D), np.float32)

    group_prep = {}
    for gi, heads in enumerate((HEADS_G0, HEADS_G1)):
        q_rows = np.concatenate([w_qkv[h * HD:(h + 1) * HD] for h in heads], 0)
        k_rows = np.concatenate(
            [w_qkv[D + h * HD:D + (h + 1) * HD] for h in heads], 0)
        v_rows = np.concatenate(
            [w_qkv[2 * D + h * HD:2 * D + (h + 1) * HD] for h in heads], 0)
        wqk_rows = np.concatenate([q_rows, k_rows], 0)
        group_prep[gi] = dict(
            wqkT=np.ascontiguousarray((wqk_rows * g[None, :]).T),
            bqk=np.ascontiguousarray((wqk_rows @ bt)[:, None]),
            wvT=np.ascontiguousarray((v_rows * g[None, :]).T),
            bv=np.ascontiguousarray((v_rows @ bt)[None, :]),
            woT=np.ascontiguousarray(np.concatenate(
                [w_out[:, h * HD:(h + 1) * HD] for h in heads], 1).T),
        )

    in_maps = []
    for c in range(N_CORES):
        b, gi = c // 2, c % 2
        p = group_prep[gi]
        in_maps.append({
            "xT": np.ascontiguousarray(x[b].T),
            "xres": x[b] if gi == 0 else zeros_res,
            "wqkT": p["wqkT"],
            "wvT": p["wvT"],
            "woT": p["woT"],
            "bqk": p["bqk"],
            "bv": p["bv"],
            "bo": b_out[None, :] if gi == 0 else zeros_bo,
            "maskT": maskT,
        })
    return in_maps


def kernel(x, adj, w_qkv, w_out, b_out, ln_gamma, ln_beta):
    from concourse.bass_utils import run_bass_kernel_spmd
    nc = _build()
    in_maps = _prep_inputs(x, adj, w_qkv, w_out, b_out, ln_gamma, ln_beta)
    res = run_bass_kernel_spmd(nc, in_maps, list(range(N_CORES)))
    out = np.empty((B, N, D), np.float32)
    for b in range(B):
        out[b] = res.results[2 * b]["y"] + res.results[2 * b + 1]["y"]
    return out
